# revision 28
# baseline (speedup 1.0000x reference)
"""Trainium2 Bass kernel for nn_AttnBlock (GroupNorm + 4-head attention + output proj).

Sharding: 8 cores = (batch b in {0,1}) x (head h in {0..3}).  Each core computes
the full attention for its (b, h) pair plus the partial output projection
wo[:, head_cols] @ att_out_head -> [512, 4096].  The host sums the 4 head
partials per batch, applies the per-query softmax denominator (computed
on-device up to the final 128-partition sum), and adds the residual x and
output bias bo (gather/unshard).

Per-core kernel (fp32 data, f32r matmuls; q/k/P/V in bf16):
  1. GroupNorm(32 groups): bn_stats per 128-channel chunk, folded into the
     projection weights (x stays raw in SBUF).
  2. Projections k, q, v -> bf16, drained on DVE.  The PE never idles waiting
     for them: window 0 interleaves the k projection with group 0's score
     matmuls; window 1 interleaves the v/q projections + PE transposes of v
     (vt [4096, 128] bf16) with group 1's scores AND group 0's AV matmuls.
  3. Per 512-query group g (S^T layout):
       S^T[j,i] = k^T q        32 bf16 matmuls [128j, 512i], pairs in PSUM
       P = exp(scale * S^T)    one ACT exp per pair -> bf16 pt tiles
       den partial             DVE pairwise add tree over the 32 bf16 tiles
                               -> [128, 512] f32, DMA'd out (host finishes the
                               128-way sum and divides during unshard)
       out^T = V P             32 accumulating bf16 matmuls, interleaved into
                               the NEXT group's score/exp window (group 7's
                               trail their own exps inside window 7)
       yp[oc] = wo_h[oc]^T (out^T as f32r)   4 matmuls, DVE copy, DMA
  Steady state keeps the PE saturated: every window carries the next group's
  scores plus the previous group's AV + output projection.
"""

import sys

sys.path.insert(0, "/opt/trn_rl_repo")

import ml_dtypes
import numpy as np

C = 512
HEADS = 4
HC = 128          # head channels
N = 4096          # h*w pixels
P = 128           # partitions
NCH = C // P      # 4 channel chunks
NJT = N // P      # 32 key tiles
IG = 512          # query-group width
NIG = N // IG     # 8 query groups
GSIZE = 16        # channels per groupnorm group
EPS = 1e-6
SCALE = float(C) ** -0.5

_NC_CACHE = {}


def _build_nc():
    from contextlib import ExitStack

    import concourse.bacc as bacc
    import concourse.bass as bass
    import concourse.tile as tile
    from concourse import mybir
    from concourse.masks import make_identity

    f32 = mybir.dt.float32
    f32r = mybir.dt.float32r
    bf16 = mybir.dt.bfloat16

    AF = mybir.ActivationFunctionType
    AX = mybir.AxisListType

    nc = bacc.Bacc("TRN2", target_bir_lowering=False, debug=False)

    xb = nc.dram_tensor("xb", [C, N], bf16, kind="ExternalInput").ap()
    wqt = nc.dram_tensor("wqt", [C, HC], f32r, kind="ExternalInput").ap()
    wkt = nc.dram_tensor("wkt", [C, HC], f32r, kind="ExternalInput").ap()
    wvt = nc.dram_tensor("wvt", [C, HC], f32r, kind="ExternalInput").ap()
    wot = nc.dram_tensor("wot", [HC, C], f32r, kind="ExternalInput").ap()
    bqh = nc.dram_tensor("bqh", [HC, 1], f32, kind="ExternalInput").ap()
    bkh = nc.dram_tensor("bkh", [HC, 1], f32, kind="ExternalInput").ap()
    bvh = nc.dram_tensor("bvh", [HC, 1], f32, kind="ExternalInput").ap()
    gns = nc.dram_tensor("gns", [1, C], f32, kind="ExternalInput").ap()
    gnb = nc.dram_tensor("gnb", [1, C], f32, kind="ExternalInput").ap()
    yp = nc.dram_tensor("yp", [C, N], f32, kind="ExternalOutput").ap()
    dden = nc.dram_tensor("dden", [P, N], f32, kind="ExternalOutput").ap()

    xbv = xb.rearrange("(a p) n -> a p n", p=P)            # [4, 128, 4096]
    wqv = wqt.rearrange("(a p) o -> p a o", p=P)           # [128, 4, 128]
    wkv = wkt.rearrange("(a p) o -> p a o", p=P)
    wvv = wvt.rearrange("(a p) o -> p a o", p=P)
    ypv = yp.rearrange("(oc p) (g i) -> oc p g i", p=P, i=IG)  # [4, 128, 8, 512]
    ddenv = dden.rearrange("p (g i) -> p g i", i=IG)       # [128, 8, 512]

    with tile.TileContext(nc) as tc, ExitStack() as ctx:
        consts = ctx.enter_context(tc.tile_pool(name="consts", bufs=1))
        qkv = ctx.enter_context(tc.tile_pool(name="qkv", bufs=1))
        pps = ctx.enter_context(tc.tile_pool(name="pps", bufs=2, space="PSUM"))

        # attention pools that must coexist with the prologue (windows 0/1)
        ptp = ctx.enter_context(tc.tile_pool(name="ptp", bufs=2))
        treep = ctx.enter_context(tc.tile_pool(name="treep", bufs=1))
        denp = ctx.enter_context(tc.tile_pool(name="denp", bufs=1))
        ppo = ctx.enter_context(tc.tile_pool(name="ppo", bufs=2, space="PSUM"))

        # prologue-scoped pools (space reclaimed before the epilogue pools open)
        pro = ExitStack()
        xpool = pro.enter_context(tc.tile_pool(name="xpool", bufs=1))
        stats = pro.enter_context(tc.tile_pool(name="stats", bufs=1))
        stats2 = pro.enter_context(tc.tile_pool(name="stats2", bufs=2))
        ppsm = pro.enter_context(tc.tile_pool(name="ppsm", bufs=2, space="PSUM"))

        # ---- constants / weights ----
        ident = consts.tile([P, P], f32)
        make_identity(nc, ident)
        eps4 = consts.tile([NCH, 1], f32)
        nc.vector.memset(eps4, EPS)
        zero1 = consts.tile([P, 1], f32)
        nc.vector.memset(zero1, 0.0)
        # GN-folded projection weights: wX_s[:, c, :] = wX[:, c, :] * A_c
        wq_s = consts.tile([P, NCH, HC], bf16)
        wk_s = consts.tile([P, NCH, HC], bf16)
        wv_s = consts.tile([P, NCH, HC], bf16)

        # ---- load x first (critical path), 8 slices per channel chunk so a
        # chunk's statistics can start as soon as that chunk's queues drain ----
        xcs = [xpool.tile([P, N], bf16, name=f"xch{i}", tag=f"xch{i}") for i in range(NCH)]
        NSL = N // 4
        for ci in range(NCH):
            for sl in range(4):
                nc.sync.dma_start(
                    out=xcs[ci][:, sl * NSL : (sl + 1) * NSL],
                    in_=xbv[ci][:, sl * NSL : (sl + 1) * NSL],
                )

        w_q = consts.tile([P, NCH, HC], f32r)
        nc.sync.dma_start(out=w_q, in_=wqv)
        w_k = consts.tile([P, NCH, HC], f32r)
        nc.sync.dma_start(out=w_k, in_=wkv)
        w_v = consts.tile([P, NCH, HC], f32r)
        nc.sync.dma_start(out=w_v, in_=wvv)
        w_o_r = consts.tile([P, C], f32r)
        nc.sync.dma_start(out=w_o_r, in_=wot)
        w_o = consts.tile([P, C], bf16)
        nc.vector.tensor_copy(out=w_o, in_=w_o_r.bitcast(f32))
        bq_sb = consts.tile([P, 1], f32)
        nc.sync.dma_start(out=bq_sb, in_=bqh)
        bk_sb = consts.tile([P, 1], f32)
        nc.sync.dma_start(out=bk_sb, in_=bkh)
        bv_sb = consts.tile([P, 1], f32)
        nc.sync.dma_start(out=bv_sb, in_=bvh)
        # gn scale/bias as two [2, 128] tiles (base partition 0) per chunk-pair
        gns_h = [consts.tile([2, P], f32, name=f"gns{h}", tag=f"gns{h}") for h in range(2)]
        gnb_h = [consts.tile([2, P], f32, name=f"gnb{h}", tag=f"gnb{h}") for h in range(2)]
        gnsv = gns.rearrange("a (b c) -> (a b) c", b=NCH)
        gnbv = gnb.rearrange("a (b c) -> (a b) c", b=NCH)
        for h in range(2):
            nc.sync.dma_start(out=gns_h[h], in_=gnsv[2 * h : 2 * h + 2, :])
            nc.sync.dma_start(out=gnb_h[h], in_=gnbv[2 * h : 2 * h + 2, :])

        # ---- GroupNorm ----
        # Every 16-channel group lives inside one 128-channel chunk, so the
        # stats -> apply chain runs per chunk-PAIR: the weight folds over
        # chunks 0/1 start while chunks 2/3 are still in bn_stats.
        mv = stats.tile([P, NCH, 2], f32)
        acol = stats.tile([P, NCH], f32)
        bcol = stats.tile([P, NCH], f32r)
        # half 0 (chunks 0/1) statistics on the otherwise-idle ACT engine:
        # Identity / Square passes with accum_out give sum(x) and sum(x^2)
        # per DMA slice; sum(x^2)/n IS the var+mean^2 the math below needs.
        as1 = stats.tile([P, 2, 4], f32)
        as2 = stats.tile([P, 2, 4], f32)
        for ci in (0, 1):
            for s in range(4):
                xsl = xcs[ci][:, s * NSL : (s + 1) * NSL]
                accsc = stats2.tile([P, NSL], bf16, name="accsc", tag="accsc")
                nc.scalar.activation(
                    out=accsc, in_=xsl, func=AF.Identity, bias=zero1,
                    accum_out=as1[:, ci, s : s + 1],
                )
                accsc = stats2.tile([P, NSL], bf16, name="accsc", tag="accsc")
                nc.scalar.activation(
                    out=accsc, in_=xsl, func=AF.Square,
                    accum_out=as2[:, ci, s : s + 1],
                )

        def gn_half(h):
            lo = 2 * h
            vpm = stats.tile([P, 2], f32, name=f"vpm{h}", tag=f"vpm{h}")
            if h == 0:
                nc.vector.reduce_sum(out=mv[:, 0:2, 0], in_=as1, axis=AX.X)
                nc.vector.tensor_scalar_mul(mv[:, 0:2, 0], mv[:, 0:2, 0], 1.0 / N)
                nc.vector.reduce_sum(out=vpm[:], in_=as2, axis=AX.X)
                nc.vector.tensor_scalar_mul(vpm, vpm, 1.0 / N)
            else:
                # per-channel mean/var for the two chunks (DVE bn_stats)
                for ci in (lo, lo + 1):
                    st = stats2.tile([P, 8, 6], f32, name="st", tag="st")
                    xv = xcs[ci][:].rearrange("p (s f) -> p s f", f=512)
                    for s in range(8):
                        nc.vector.bn_stats(out=st[:, s, :], in_=xv[:, s, :])
                    nc.vector.bn_aggr(out=mv[:, ci, :], in_=st)
                # vpm = var + mean^2
                nc.vector.tensor_mul(vpm, mv[:, lo : lo + 2, 0], mv[:, lo : lo + 2, 0])
                nc.vector.tensor_add(vpm, vpm, mv[:, lo : lo + 2, 1])
            # transpose to chunk-major rows [2, 128]
            mrow = stats.tile([2, P], f32, name=f"mrow{h}", tag=f"mrow{h}")
            vrow = stats.tile([2, P], f32, name=f"vrow{h}", tag=f"vrow{h}")
            pmz = ppsm.tile([2, P], f32, name="pmz", tag="sm")
            nc.tensor.transpose(pmz, mv[:, lo : lo + 2, 0], ident)
            nc.vector.tensor_copy(out=mrow, in_=pmz)
            pvz = ppsm.tile([2, P], f32, name="pvz", tag="sm")
            nc.tensor.transpose(pvz, vpm, ident)
            nc.vector.tensor_copy(out=vrow, in_=pvz)
            # group means -> [2, 8]
            gm = stats.tile([2, 8], f32, name=f"gm{h}", tag=f"gm{h}")
            gv = stats.tile([2, 8], f32, name=f"gv{h}", tag=f"gv{h}")
            nc.vector.reduce_sum(
                out=gm[:], in_=mrow[:].rearrange("p (g s) -> p g s", s=GSIZE), axis=AX.X
            )
            nc.vector.tensor_scalar_mul(gm, gm, 1.0 / GSIZE)
            nc.vector.reduce_sum(
                out=gv[:], in_=vrow[:].rearrange("p (g s) -> p g s", s=GSIZE), axis=AX.X
            )
            nc.vector.tensor_scalar_mul(gv, gv, 1.0 / GSIZE)
            gmsq = stats.tile([2, 8], f32, name=f"gmsq{h}", tag=f"gmsq{h}")
            nc.vector.tensor_mul(gmsq, gm, gm)
            nc.vector.tensor_sub(gv, gv, gmsq)     # group variance
            nc.scalar.activation(out=gv, in_=gv, func=AF.Sqrt, bias=eps4[0:2, :])
            nc.vector.reciprocal(gv, gv)           # rstd per group
            # expand groups to channels [2, 128]
            grx = stats.tile([2, P], f32, name=f"grx{h}", tag=f"grx{h}")
            gmx = stats.tile([2, P], f32, name=f"gmx{h}", tag=f"gmx{h}")
            gv_ap = gv[:]
            gm_ap = gm[:]
            gv_b = bass.AP(tensor=gv_ap.tensor, offset=gv_ap.offset, ap=list(gv_ap.ap) + [[0, GSIZE]])
            gm_b = bass.AP(tensor=gm_ap.tensor, offset=gm_ap.offset, ap=list(gm_ap.ap) + [[0, GSIZE]])
            nc.vector.tensor_copy(out=grx[:].rearrange("p (g s) -> p g s", s=GSIZE), in_=gv_b)
            nc.vector.tensor_copy(out=gmx[:].rearrange("p (g s) -> p g s", s=GSIZE), in_=gm_b)
            nc.vector.tensor_mul(grx, grx, gns_h[h])
            nc.vector.tensor_mul(gmx, gmx, grx)
            nc.vector.tensor_sub(gmx, gnb_h[h], gmx)
            # back to per-partition scalars [128, 2]
            paz = ppsm.tile([P, 2], f32, name="paz", tag="sm")
            nc.tensor.transpose(paz, grx, ident[0:2, 0:2])
            nc.vector.tensor_copy(out=acol[:, lo : lo + 2], in_=paz)
            pbz = ppsm.tile([P, 2], f32, name="pbz", tag="sm")
            nc.tensor.transpose(pbz, gmx, ident[0:2, 0:2])
            nc.vector.tensor_copy(out=bcol[:, lo : lo + 2], in_=pbz)
            # fold GN into the projection weights instead of rewriting x:
            # wX_s[:, ci, :] = wX[:, ci, :] * A_ci   (tiny ACT ops; x stays raw)
            for ci in (lo, lo + 1):
                for wsrc, wdst in ((w_q, wq_s), (w_k, wk_s), (w_v, wv_s)):
                    nc.scalar.activation(
                        out=wdst[:, ci, :],
                        in_=wsrc[:, ci, :].bitcast(f32),
                        func=AF.Identity,
                        bias=zero1,
                        scale=acol[:, ci : ci + 1],
                    )

        gn_half(0)
        gn_half(1)

        # ---- projections (bf16 outputs, DVE drains) ----
        q_sb = qkv.tile([P, N], bf16)
        k_sb = qkv.tile([P, N], bf16)
        vt_sb = qkv.tile([P, NJT, HC], bf16)
        v_sb = xpool.tile([P, N], bf16)

        def make_b2(w_raw, b_sb, name):
            # bias fold: bvec = W^T B  (per output channel), added to the conv bias
            pbv = ppsm.tile([P, 1], f32, name="pbv", tag="sm")
            for ci in range(NCH):
                nc.tensor.matmul(
                    pbv,
                    lhsT=w_raw[:, ci, :].bitcast(f32),
                    rhs=bcol[:, ci : ci + 1].bitcast(f32),
                    start=(ci == 0),
                    stop=(ci == NCH - 1),
                )
            b2 = stats.tile([P, 1], f32, name=name, tag=name)
            nc.vector.tensor_add(b2, b_sb, pbv)
            return b2

        b2k = make_b2(w_k, bk_sb, "b2k")
        b2q = make_b2(w_q, bq_sb, "b2q")
        b2v = make_b2(w_v, bv_sb, "b2v")

        def proj_group(w_sb, b2, dst, nh, drain="dve"):
            # one 512-pixel projection group: 4 accumulating matmuls + drain.
            # PSUM comes from ppsm so the score pairs keep pps to themselves.
            pp = ppsm.tile([P, IG], f32, name="pp", tag="sm")
            for ci in range(NCH):
                nc.tensor.matmul(
                    pp,
                    lhsT=w_sb[:, ci, :],
                    rhs=xcs[ci][:, nh * IG : (nh + 1) * IG],
                    start=(ci == 0),
                    stop=(ci == NCH - 1),
                )
            dsl = dst[:, nh * IG : (nh + 1) * IG]
            if drain == "act":
                nc.scalar.activation(out=dsl, in_=pp, func=AF.Identity, bias=b2, scale=1.0)
            else:
                nc.vector.tensor_scalar_add(out=dsl, in0=pp, scalar1=b2)

        # ---- attention machinery ----
        pt_tiles = [None] * NIG
        po_tiles = [None] * NIG

        def issue_spair(g, jp):
            qs = q_sb[:, g * IG : (g + 1) * IG]
            ps = pps.tile([P, 2, IG], f32, tag="ps")
            for h in range(2):
                jt = 2 * jp + h
                nc.tensor.matmul(
                    ps[:, h, :],
                    lhsT=k_sb[:, jt * P : (jt + 1) * P],
                    rhs=qs,
                    start=True,
                    stop=True,
                )
            nc.scalar.activation(
                out=pt_tiles[g][:, 2 * jp : 2 * jp + 2, :],
                in_=ps,
                func=AF.Exp,
                scale=SCALE,
            )

        def issue_av(g, jt):
            nc.tensor.matmul(
                po_tiles[g],
                lhsT=vt_sb[:, jt, :],
                rhs=pt_tiles[g][:, jt, :],
                start=(jt == 0),
                stop=(jt == NJT - 1),
            )

        def issue_tree_levels(g, scr):
            # levels 2..5 of the den tree (distinct scratch ranges)
            v8 = scr[:, 0:16, :].rearrange("p (a b) i -> p a b i", b=2)
            nc.vector.tensor_add(scr[:, 16:24, :], v8[:, :, 0, :], v8[:, :, 1, :])
            v4 = scr[:, 16:24, :].rearrange("p (a b) i -> p a b i", b=2)
            nc.vector.tensor_add(scr[:, 0:4, :], v4[:, :, 0, :], v4[:, :, 1, :])
            v2 = scr[:, 0:4, :].rearrange("p (a b) i -> p a b i", b=2)
            nc.vector.tensor_add(scr[:, 4:6, :], v2[:, :, 0, :], v2[:, :, 1, :])
            denf = denp.tile([P, IG], f32, tag="denf")
            nc.vector.tensor_add(denf, scr[:, 4, :], scr[:, 5, :])
            nc.sync.dma_start(out=ddenv[:, g, :], in_=denf)

        def issue_tree(g):
            # pairwise add tree over the 32 bf16 P tiles -> den partial [128, 512]
            ptv = pt_tiles[g][:]
            v16 = ptv.rearrange("p (a b) i -> p a b i", b=2)
            scr = treep.tile([P, 24, IG], bf16, name="scr", tag="scr")
            nc.vector.tensor_add(scr[:, 0:16, :], v16[:, :, 0, :], v16[:, :, 1, :])
            issue_tree_levels(g, scr)

        def issue_ot_op(g, otp, yfp):
            ot = otp.tile([P, IG], bf16, tag="ot")
            nc.vector.tensor_copy(out=ot, in_=po_tiles[g])
            for oc in range(NCH):
                pf = ppf.tile([P, IG], f32, tag="pf")
                nc.tensor.matmul(
                    pf, lhsT=w_o[:, oc * P : (oc + 1) * P], rhs=ot, start=True, stop=True
                )
                yf = yfp.tile([P, IG], f32, tag="yf")
                nc.vector.tensor_copy(out=yf, in_=pf)
                nc.sync.dma_start(out=ypv[oc, :, g, :], in_=yf)

        # ---- window 0: k projection interleaved with group-0 scores ----
        # (k-proj of block nh+2 is issued before the scores that consume block
        #  nh, so the PE never waits on a DVE drain.)
        pt_tiles[0] = ptp.tile([P, NJT, IG], bf16, name="pt0", tag="pt")
        proj_group(wq_s, b2q, q_sb, 0, drain="act")
        proj_group(wk_s, b2k, k_sb, 0, drain="act")
        proj_group(wk_s, b2k, k_sb, 1, drain="act")
        for nh in range(NIG):
            if nh + 2 < NIG:
                proj_group(wk_s, b2k, k_sb, nh + 2, drain="act")
            issue_spair(0, 2 * nh)
            issue_spair(0, 2 * nh + 1)
        issue_tree(0)

        # ---- window 1: v/q projections, v transposes and group-0 AV,
        # interleaved with group-1 scores ----
        pt_tiles[1] = ptp.tile([P, NJT, IG], bf16, name="pt1", tag="pt")
        po_tiles[0] = ppo.tile([P, IG], f32, name="po0", tag="po")
        proj_group(wq_s, b2q, q_sb, 1)

        def dma_tr(t):
            # XBAR DMA transpose: vt[p, 8t+u, c] = v^T[(8t+u)*128 + p, c]
            nc.sync.dma_start_transpose(
                out=vt_sb[:, 8 * t : 8 * (t + 1), :],
                in_=v_sb[:, 1024 * t : 1024 * (t + 1)],
            )

        work = []
        for t in range(4):
            work.append(("proj", (wv_s, b2v, v_sb, 2 * t)))
            work.append(("proj", (wv_s, b2v, v_sb, 2 * t + 1)))
            work.append(("tr", t))
            if t >= 1:
                work += [("av", jt) for jt in range(8 * (t - 1), 8 * t)]
        work.append(("proj", (wq_s, b2q, q_sb, 2)))
        work.append(("proj", (wq_s, b2q, q_sb, 3)))
        work += [("av", jt) for jt in range(24, 32)]
        work += [("proj", (wq_s, b2q, q_sb, nh)) for nh in range(4, NIG)]
        wi = 0
        for jp in range(NJT // 2):
            issue_spair(1, jp)
            quota = (len(work) * (jp + 1)) // (NJT // 2)
            while wi < quota:
                kind, arg = work[wi]
                wi += 1
                if kind == "proj":
                    proj_group(*arg)
                elif kind == "tr":
                    dma_tr(arg)
                else:
                    issue_av(0, arg)
        issue_tree(1)

        pro.close()

        # epilogue-only pools (opened after x / the prologue PSUM are freed)
        otp = ctx.enter_context(tc.tile_pool(name="otp", bufs=2))
        yfp = ctx.enter_context(tc.tile_pool(name="yfp", bufs=2))
        ppf = ctx.enter_context(tc.tile_pool(name="ppf", bufs=2, space="PSUM"))

        issue_ot_op(0, otp, yfp)

        # ---- windows 2..7: scores(g) + AV(g-1) + output projection(g-1).
        # Group 7's AV additionally trails its own exps inside window 7. ----
        for g in range(2, NIG):
            pt_tiles[g] = ptp.tile([P, NJT, IG], bf16, name=f"pt{g}", tag="pt")
            po_tiles[g - 1] = ppo.tile([P, IG], f32, name=f"po{g-1}", tag="po")
            self_trail = g == NIG - 1
            if self_trail:
                po_tiles[g] = ppo.tile([P, IG], f32, name=f"po{g}", tag="po")
            for jp in range(NJT // 2):
                issue_spair(g, jp)
                issue_av(g - 1, 2 * jp)
                issue_av(g - 1, 2 * jp + 1)
                if self_trail and jp >= 2:
                    issue_av(g, 2 * (jp - 2))
                    issue_av(g, 2 * (jp - 2) + 1)
            if g == NIG - 1:
                # first half of the last group's den tree level 1 (tiles 0..15
                # are ready mid-window, so this costs nothing at the end)
                scr7 = treep.tile([P, 24, IG], bf16, name="scr7", tag="scr")
                vh = pt_tiles[g][:, 0:16, :].rearrange("p (a b) i -> p a b i", b=2)
                nc.vector.tensor_add(scr7[:, 0:8, :], vh[:, :, 0, :], vh[:, :, 1, :])
            issue_ot_op(g - 1, otp, yfp)
            if g < NIG - 1:
                issue_tree(g)

        # tail: last 4 AV tiles of group 7, the rest of its den tree level 1
        # (kept off the yf-copy path), its output projection, then the
        # remaining tree levels
        g = NIG - 1
        for jt in range(NJT - 4, NJT):
            issue_av(g, jt)
        vh = pt_tiles[g][:, 16:32, :].rearrange("p (a b) i -> p a b i", b=2)
        nc.vector.tensor_add(scr7[:, 8:16, :], vh[:, :, 0, :], vh[:, :, 1, :])
        issue_ot_op(g, otp, yfp)
        issue_tree_levels(g, scr7)

    nc.compile()
    return nc


def get_nc():
    if "nc" not in _NC_CACHE:
        _NC_CACHE["nc"] = _build_nc()
    return _NC_CACHE["nc"]


def make_in_maps(inputs):
    x = np.ascontiguousarray(np.asarray(inputs["x"], dtype=np.float32))
    wq = np.asarray(inputs["wq"], np.float32)
    wk = np.asarray(inputs["wk"], np.float32)
    wv = np.asarray(inputs["wv"], np.float32)
    bq = np.asarray(inputs["bq"], np.float32)
    bk = np.asarray(inputs["bk"], np.float32)
    bv = np.asarray(inputs["bv"], np.float32)
    wo = np.asarray(inputs["wo"], np.float32)
    gn_scale = np.asarray(inputs["gn_scale"], np.float32)
    gn_bias = np.asarray(inputs["gn_bias"], np.float32)

    in_maps = []
    for cid in range(8):
        b, h = divmod(cid, HEADS)
        sl = slice(h * HC, (h + 1) * HC)
        in_maps.append(
            {
                "xb": np.ascontiguousarray(x[b].reshape(C, N).astype(ml_dtypes.bfloat16)),
                "wqt": np.ascontiguousarray(wq[sl, :].T),
                "wkt": np.ascontiguousarray(wk[sl, :].T),
                "wvt": np.ascontiguousarray(wv[sl, :].T),
                "wot": np.ascontiguousarray(wo[:, sl].T),
                "bqh": np.ascontiguousarray(bq[sl].reshape(HC, 1)),
                "bkh": np.ascontiguousarray(bk[sl].reshape(HC, 1)),
                "bvh": np.ascontiguousarray(bv[sl].reshape(HC, 1)),
                "gns": np.ascontiguousarray(gn_scale.reshape(1, C)),
                "gnb": np.ascontiguousarray(gn_bias.reshape(1, C)),
            }
        )
    return in_maps


def assemble_output(inputs, results):
    x = np.asarray(inputs["x"], np.float32)
    bo = np.asarray(inputs["bo"], np.float32)
    y = x.reshape(2, C, N).astype(np.float32).copy()
    y += bo.reshape(1, C, 1)
    for cid in range(8):
        b = cid // HEADS
        r = results[cid]
        den = np.asarray(r["dden"], np.float32).sum(axis=0)  # [N]
        y[b] += np.asarray(r["yp"], np.float32) * (1.0 / den)[None, :]
    return y.reshape(2, C, 64, 64)


def run(inputs, trace=False):
    from concourse.bass_utils import run_bass_kernel_spmd

    nc = get_nc()
    in_maps = make_in_maps(inputs)
    res = run_bass_kernel_spmd(nc, in_maps, list(range(8)), trace=trace)
    return assemble_output(inputs, res.results), res


def kernel(**inputs):
    y, _ = run(inputs, trace=False)
    return y


# revision 29
# speedup vs baseline: 1.0113x; 1.0113x over previous
"""Trainium2 Bass kernel for nn_AttnBlock (GroupNorm + 4-head attention + output proj).

Sharding: 8 cores = (batch b in {0,1}) x (head h in {0..3}).  Each core computes
the full attention for its (b, h) pair plus the partial output projection
wo[:, head_cols] @ att_out_head -> [512, 4096].  The host sums the 4 head
partials per batch, applies the per-query softmax denominator (computed
on-device up to the final 128-partition sum), and adds the residual x and
output bias bo (gather/unshard).

Per-core kernel (fp32 data, f32r matmuls; q/k/P/V in bf16):
  1. GroupNorm(32 groups): bn_stats per 128-channel chunk, folded into the
     projection weights (x stays raw in SBUF).
  2. Projections k, q, v -> bf16, drained on DVE.  The PE never idles waiting
     for them: window 0 interleaves the k projection with group 0's score
     matmuls; window 1 interleaves the v/q projections + PE transposes of v
     (vt [4096, 128] bf16) with group 1's scores AND group 0's AV matmuls.
  3. Per 512-query group g (S^T layout):
       S^T[j,i] = k^T q        32 bf16 matmuls [128j, 512i], pairs in PSUM
       P = exp(scale * S^T)    one ACT exp per pair -> bf16 pt tiles
       den partial             DVE pairwise add tree over the 32 bf16 tiles
                               -> [128, 512] f32, DMA'd out (host finishes the
                               128-way sum and divides during unshard)
       out^T = V P             32 accumulating bf16 matmuls, interleaved into
                               the NEXT group's score/exp window (group 7's
                               trail their own exps inside window 7)
       yp[oc] = wo_h[oc]^T (out^T as f32r)   4 matmuls, DVE copy, DMA
  Steady state keeps the PE saturated: every window carries the next group's
  scores plus the previous group's AV + output projection.
"""

import sys

sys.path.insert(0, "/opt/trn_rl_repo")

import ml_dtypes
import numpy as np

C = 512
HEADS = 4
HC = 128          # head channels
N = 4096          # h*w pixels
P = 128           # partitions
NCH = C // P      # 4 channel chunks
NJT = N // P      # 32 key tiles
IG = 512          # query-group width
NIG = N // IG     # 8 query groups
GSIZE = 16        # channels per groupnorm group
EPS = 1e-6
SCALE = float(C) ** -0.5

_NC_CACHE = {}


def _build_nc():
    from contextlib import ExitStack

    import concourse.bacc as bacc
    import concourse.bass as bass
    import concourse.tile as tile
    from concourse import mybir
    from concourse.masks import make_identity

    f32 = mybir.dt.float32
    f32r = mybir.dt.float32r
    bf16 = mybir.dt.bfloat16

    AF = mybir.ActivationFunctionType
    AX = mybir.AxisListType

    nc = bacc.Bacc("TRN2", target_bir_lowering=False, debug=False)

    xb = nc.dram_tensor("xb", [C, N], bf16, kind="ExternalInput").ap()
    wqt = nc.dram_tensor("wqt", [C, HC], f32r, kind="ExternalInput").ap()
    wkt = nc.dram_tensor("wkt", [C, HC], f32r, kind="ExternalInput").ap()
    wvt = nc.dram_tensor("wvt", [C, HC], f32r, kind="ExternalInput").ap()
    wot = nc.dram_tensor("wot", [HC, C], f32r, kind="ExternalInput").ap()
    bqh = nc.dram_tensor("bqh", [HC, 1], f32, kind="ExternalInput").ap()
    bkh = nc.dram_tensor("bkh", [HC, 1], f32, kind="ExternalInput").ap()
    bvh = nc.dram_tensor("bvh", [HC, 1], f32, kind="ExternalInput").ap()
    gns = nc.dram_tensor("gns", [1, C], f32, kind="ExternalInput").ap()
    gnb = nc.dram_tensor("gnb", [1, C], f32, kind="ExternalInput").ap()
    yp = nc.dram_tensor("yp", [C, N], f32, kind="ExternalOutput").ap()
    dden = nc.dram_tensor("dden", [P, N], f32, kind="ExternalOutput").ap()

    xbv = xb.rearrange("(a p) n -> a p n", p=P)            # [4, 128, 4096]
    wqv = wqt.rearrange("(a p) o -> p a o", p=P)           # [128, 4, 128]
    wkv = wkt.rearrange("(a p) o -> p a o", p=P)
    wvv = wvt.rearrange("(a p) o -> p a o", p=P)
    ypv = yp.rearrange("(oc p) (g i) -> oc p g i", p=P, i=IG)  # [4, 128, 8, 512]
    ddenv = dden.rearrange("p (g i) -> p g i", i=IG)       # [128, 8, 512]

    with tile.TileContext(nc) as tc, ExitStack() as ctx:
        consts = ctx.enter_context(tc.tile_pool(name="consts", bufs=1))
        qkv = ctx.enter_context(tc.tile_pool(name="qkv", bufs=1))
        pps = ctx.enter_context(tc.tile_pool(name="pps", bufs=2, space="PSUM"))

        # attention pools that must coexist with the prologue (windows 0/1)
        ptp = ctx.enter_context(tc.tile_pool(name="ptp", bufs=2))
        treep = ctx.enter_context(tc.tile_pool(name="treep", bufs=1))
        denp = ctx.enter_context(tc.tile_pool(name="denp", bufs=1))
        ppo = ctx.enter_context(tc.tile_pool(name="ppo", bufs=2, space="PSUM"))

        # prologue-scoped pools (space reclaimed before the epilogue pools open)
        pro = ExitStack()
        xpool = pro.enter_context(tc.tile_pool(name="xpool", bufs=1))
        stats = pro.enter_context(tc.tile_pool(name="stats", bufs=1))
        stats2 = pro.enter_context(tc.tile_pool(name="stats2", bufs=2))
        ppsm = pro.enter_context(tc.tile_pool(name="ppsm", bufs=2, space="PSUM"))

        # ---- constants / weights ----
        ident = consts.tile([P, P], f32)
        make_identity(nc, ident)
        eps4 = consts.tile([NCH, 1], f32)
        nc.vector.memset(eps4, EPS)
        zero1 = consts.tile([P, 1], f32)
        nc.vector.memset(zero1, 0.0)
        # GN-folded projection weights: wX_s[:, c, :] = wX[:, c, :] * A_c
        wq_s = consts.tile([P, NCH, HC], bf16)
        wk_s = consts.tile([P, NCH, HC], bf16)
        wv_s = consts.tile([P, NCH, HC], bf16)

        # ---- load x first (critical path), 8 slices per channel chunk so a
        # chunk's statistics can start as soon as that chunk's queues drain ----
        xcs = [xpool.tile([P, N], bf16, name=f"xch{i}", tag=f"xch{i}") for i in range(NCH)]
        NSL = N // 4
        for ci in range(NCH):
            for sl in range(4):
                nc.sync.dma_start(
                    out=xcs[ci][:, sl * NSL : (sl + 1) * NSL],
                    in_=xbv[ci][:, sl * NSL : (sl + 1) * NSL],
                )

        w_q = consts.tile([P, NCH, HC], f32r)
        nc.sync.dma_start(out=w_q, in_=wqv)
        w_k = consts.tile([P, NCH, HC], f32r)
        nc.sync.dma_start(out=w_k, in_=wkv)
        w_v = consts.tile([P, NCH, HC], f32r)
        nc.sync.dma_start(out=w_v, in_=wvv)
        w_o = consts.tile([P, C], f32r)
        nc.sync.dma_start(out=w_o, in_=wot)
        bq_sb = consts.tile([P, 1], f32)
        nc.sync.dma_start(out=bq_sb, in_=bqh)
        bk_sb = consts.tile([P, 1], f32)
        nc.sync.dma_start(out=bk_sb, in_=bkh)
        bv_sb = consts.tile([P, 1], f32)
        nc.sync.dma_start(out=bv_sb, in_=bvh)
        # gn scale/bias as two [2, 128] tiles (base partition 0) per chunk-pair
        gns_h = [consts.tile([2, P], f32, name=f"gns{h}", tag=f"gns{h}") for h in range(2)]
        gnb_h = [consts.tile([2, P], f32, name=f"gnb{h}", tag=f"gnb{h}") for h in range(2)]
        gnsv = gns.rearrange("a (b c) -> (a b) c", b=NCH)
        gnbv = gnb.rearrange("a (b c) -> (a b) c", b=NCH)
        for h in range(2):
            nc.sync.dma_start(out=gns_h[h], in_=gnsv[2 * h : 2 * h + 2, :])
            nc.sync.dma_start(out=gnb_h[h], in_=gnbv[2 * h : 2 * h + 2, :])

        # ---- GroupNorm ----
        # Every 16-channel group lives inside one 128-channel chunk, so the
        # stats -> apply chain runs per chunk-PAIR: the weight folds over
        # chunks 0/1 start while chunks 2/3 are still in bn_stats.
        mv = stats.tile([P, NCH, 2], f32)
        acol = stats.tile([P, NCH], f32)
        bcol = stats.tile([P, NCH], f32r)

        def gn_half(h):
            lo = 2 * h
            # per-channel mean/var for the two chunks
            for ci in (lo, lo + 1):
                st = stats2.tile([P, 8, 6], f32, name="st", tag="st")
                xv = xcs[ci][:].rearrange("p (s f) -> p s f", f=512)
                for s in range(8):
                    nc.vector.bn_stats(out=st[:, s, :], in_=xv[:, s, :])
                nc.vector.bn_aggr(out=mv[:, ci, :], in_=st)
            # vpm = var + mean^2
            vpm = stats.tile([P, 2], f32, name=f"vpm{h}", tag=f"vpm{h}")
            nc.vector.tensor_mul(vpm, mv[:, lo : lo + 2, 0], mv[:, lo : lo + 2, 0])
            nc.vector.tensor_add(vpm, vpm, mv[:, lo : lo + 2, 1])
            # transpose to chunk-major rows [2, 128]
            mrow = stats.tile([2, P], f32, name=f"mrow{h}", tag=f"mrow{h}")
            vrow = stats.tile([2, P], f32, name=f"vrow{h}", tag=f"vrow{h}")
            pmz = ppsm.tile([2, P], f32, name="pmz", tag="sm")
            nc.tensor.transpose(pmz, mv[:, lo : lo + 2, 0], ident)
            nc.vector.tensor_copy(out=mrow, in_=pmz)
            pvz = ppsm.tile([2, P], f32, name="pvz", tag="sm")
            nc.tensor.transpose(pvz, vpm, ident)
            nc.vector.tensor_copy(out=vrow, in_=pvz)
            # group means -> [2, 8]
            gm = stats.tile([2, 8], f32, name=f"gm{h}", tag=f"gm{h}")
            gv = stats.tile([2, 8], f32, name=f"gv{h}", tag=f"gv{h}")
            nc.vector.reduce_sum(
                out=gm[:], in_=mrow[:].rearrange("p (g s) -> p g s", s=GSIZE), axis=AX.X
            )
            nc.vector.tensor_scalar_mul(gm, gm, 1.0 / GSIZE)
            nc.vector.reduce_sum(
                out=gv[:], in_=vrow[:].rearrange("p (g s) -> p g s", s=GSIZE), axis=AX.X
            )
            nc.vector.tensor_scalar_mul(gv, gv, 1.0 / GSIZE)
            gmsq = stats.tile([2, 8], f32, name=f"gmsq{h}", tag=f"gmsq{h}")
            nc.vector.tensor_mul(gmsq, gm, gm)
            nc.vector.tensor_sub(gv, gv, gmsq)     # group variance
            nc.scalar.activation(out=gv, in_=gv, func=AF.Sqrt, bias=eps4[0:2, :])
            nc.vector.reciprocal(gv, gv)           # rstd per group
            # expand groups to channels [2, 128]
            grx = stats.tile([2, P], f32, name=f"grx{h}", tag=f"grx{h}")
            gmx = stats.tile([2, P], f32, name=f"gmx{h}", tag=f"gmx{h}")
            gv_ap = gv[:]
            gm_ap = gm[:]
            gv_b = bass.AP(tensor=gv_ap.tensor, offset=gv_ap.offset, ap=list(gv_ap.ap) + [[0, GSIZE]])
            gm_b = bass.AP(tensor=gm_ap.tensor, offset=gm_ap.offset, ap=list(gm_ap.ap) + [[0, GSIZE]])
            nc.vector.tensor_copy(out=grx[:].rearrange("p (g s) -> p g s", s=GSIZE), in_=gv_b)
            nc.vector.tensor_copy(out=gmx[:].rearrange("p (g s) -> p g s", s=GSIZE), in_=gm_b)
            nc.vector.tensor_mul(grx, grx, gns_h[h])
            nc.vector.tensor_mul(gmx, gmx, grx)
            nc.vector.tensor_sub(gmx, gnb_h[h], gmx)
            # back to per-partition scalars [128, 2]
            paz = ppsm.tile([P, 2], f32, name="paz", tag="sm")
            nc.tensor.transpose(paz, grx, ident[0:2, 0:2])
            nc.vector.tensor_copy(out=acol[:, lo : lo + 2], in_=paz)
            pbz = ppsm.tile([P, 2], f32, name="pbz", tag="sm")
            nc.tensor.transpose(pbz, gmx, ident[0:2, 0:2])
            nc.vector.tensor_copy(out=bcol[:, lo : lo + 2], in_=pbz)
            # fold GN into the projection weights instead of rewriting x:
            # wX_s[:, ci, :] = wX[:, ci, :] * A_ci   (tiny ACT ops; x stays raw)
            for ci in (lo, lo + 1):
                for wsrc, wdst in ((w_q, wq_s), (w_k, wk_s), (w_v, wv_s)):
                    nc.scalar.activation(
                        out=wdst[:, ci, :],
                        in_=wsrc[:, ci, :].bitcast(f32),
                        func=AF.Identity,
                        bias=zero1,
                        scale=acol[:, ci : ci + 1],
                    )

        gn_half(0)
        gn_half(1)

        # ---- projections (bf16 outputs, DVE drains) ----
        q_sb = qkv.tile([P, N], bf16)
        k_sb = qkv.tile([P, N], bf16)
        vt_sb = qkv.tile([P, NJT, HC], bf16)
        v_sb = xpool.tile([P, N], bf16)

        def make_b2(w_raw, b_sb, name):
            # bias fold: bvec = W^T B  (per output channel), added to the conv bias
            pbv = ppsm.tile([P, 1], f32, name="pbv", tag="sm")
            for ci in range(NCH):
                nc.tensor.matmul(
                    pbv,
                    lhsT=w_raw[:, ci, :].bitcast(f32),
                    rhs=bcol[:, ci : ci + 1].bitcast(f32),
                    start=(ci == 0),
                    stop=(ci == NCH - 1),
                )
            b2 = stats.tile([P, 1], f32, name=name, tag=name)
            nc.vector.tensor_add(b2, b_sb, pbv)
            return b2

        b2k = make_b2(w_k, bk_sb, "b2k")
        b2q = make_b2(w_q, bq_sb, "b2q")
        b2v = make_b2(w_v, bv_sb, "b2v")

        def proj_group(w_sb, b2, dst, nh, drain="dve"):
            # one 512-pixel projection group: 4 accumulating matmuls + drain.
            # PSUM comes from ppsm so the score pairs keep pps to themselves.
            pp = ppsm.tile([P, IG], f32, name="pp", tag="sm")
            for ci in range(NCH):
                nc.tensor.matmul(
                    pp,
                    lhsT=w_sb[:, ci, :],
                    rhs=xcs[ci][:, nh * IG : (nh + 1) * IG],
                    start=(ci == 0),
                    stop=(ci == NCH - 1),
                )
            dsl = dst[:, nh * IG : (nh + 1) * IG]
            if drain == "act":
                nc.scalar.activation(out=dsl, in_=pp, func=AF.Identity, bias=b2, scale=1.0)
            else:
                nc.vector.tensor_scalar_add(out=dsl, in0=pp, scalar1=b2)

        # ---- attention machinery ----
        pt_tiles = [None] * NIG
        po_tiles = [None] * NIG

        def issue_spair(g, jp):
            qs = q_sb[:, g * IG : (g + 1) * IG]
            ps = pps.tile([P, 2, IG], f32, tag="ps")
            for h in range(2):
                jt = 2 * jp + h
                nc.tensor.matmul(
                    ps[:, h, :],
                    lhsT=k_sb[:, jt * P : (jt + 1) * P],
                    rhs=qs,
                    start=True,
                    stop=True,
                )
            nc.scalar.activation(
                out=pt_tiles[g][:, 2 * jp : 2 * jp + 2, :],
                in_=ps,
                func=AF.Exp,
                scale=SCALE,
            )

        def issue_av(g, jt):
            nc.tensor.matmul(
                po_tiles[g],
                lhsT=vt_sb[:, jt, :],
                rhs=pt_tiles[g][:, jt, :],
                start=(jt == 0),
                stop=(jt == NJT - 1),
            )

        def issue_tree_levels(g, scr):
            # levels 2..5 of the den tree (distinct scratch ranges)
            v8 = scr[:, 0:16, :].rearrange("p (a b) i -> p a b i", b=2)
            nc.vector.tensor_add(scr[:, 16:24, :], v8[:, :, 0, :], v8[:, :, 1, :])
            v4 = scr[:, 16:24, :].rearrange("p (a b) i -> p a b i", b=2)
            nc.vector.tensor_add(scr[:, 0:4, :], v4[:, :, 0, :], v4[:, :, 1, :])
            v2 = scr[:, 0:4, :].rearrange("p (a b) i -> p a b i", b=2)
            nc.vector.tensor_add(scr[:, 4:6, :], v2[:, :, 0, :], v2[:, :, 1, :])
            denf = denp.tile([P, IG], f32, tag="denf")
            nc.vector.tensor_add(denf, scr[:, 4, :], scr[:, 5, :])
            nc.sync.dma_start(out=ddenv[:, g, :], in_=denf)

        def issue_tree(g):
            # pairwise add tree over the 32 bf16 P tiles -> den partial [128, 512]
            ptv = pt_tiles[g][:]
            v16 = ptv.rearrange("p (a b) i -> p a b i", b=2)
            scr = treep.tile([P, 24, IG], bf16, name="scr", tag="scr")
            nc.vector.tensor_add(scr[:, 0:16, :], v16[:, :, 0, :], v16[:, :, 1, :])
            issue_tree_levels(g, scr)

        def issue_ot_op(g, otp, yfp):
            ot = otp.tile([P, IG], f32r, tag="ot")
            nc.vector.tensor_copy(out=ot, in_=po_tiles[g])
            for oc in range(NCH):
                pf = ppf.tile([P, IG], f32, tag="pf")
                nc.tensor.matmul(
                    pf, lhsT=w_o[:, oc * P : (oc + 1) * P], rhs=ot, start=True, stop=True
                )
                yf = yfp.tile([P, IG], f32, tag="yf")
                nc.vector.tensor_copy(out=yf, in_=pf)
                nc.sync.dma_start(out=ypv[oc, :, g, :], in_=yf)

        # ---- window 0: k projection interleaved with group-0 scores ----
        # (k-proj of block nh+2 is issued before the scores that consume block
        #  nh, so the PE never waits on a DVE drain.)
        pt_tiles[0] = ptp.tile([P, NJT, IG], bf16, name="pt0", tag="pt")
        proj_group(wq_s, b2q, q_sb, 0, drain="act")
        proj_group(wk_s, b2k, k_sb, 0, drain="act")
        proj_group(wk_s, b2k, k_sb, 1, drain="act")
        for nh in range(NIG):
            if nh + 2 < NIG:
                proj_group(wk_s, b2k, k_sb, nh + 2, drain="act")
            issue_spair(0, 2 * nh)
            issue_spair(0, 2 * nh + 1)
        issue_tree(0)

        # ---- window 1: v/q projections, v transposes and group-0 AV,
        # interleaved with group-1 scores ----
        pt_tiles[1] = ptp.tile([P, NJT, IG], bf16, name="pt1", tag="pt")
        po_tiles[0] = ppo.tile([P, IG], f32, name="po0", tag="po")
        proj_group(wq_s, b2q, q_sb, 1)

        def dma_tr(t):
            # XBAR DMA transpose: vt[p, 8t+u, c] = v^T[(8t+u)*128 + p, c]
            nc.sync.dma_start_transpose(
                out=vt_sb[:, 8 * t : 8 * (t + 1), :],
                in_=v_sb[:, 1024 * t : 1024 * (t + 1)],
            )

        work = []
        for t in range(4):
            work.append(("proj", (wv_s, b2v, v_sb, 2 * t)))
            work.append(("proj", (wv_s, b2v, v_sb, 2 * t + 1)))
            work.append(("tr", t))
            if t >= 1:
                work += [("av", jt) for jt in range(8 * (t - 1), 8 * t)]
        work.append(("proj", (wq_s, b2q, q_sb, 2)))
        work.append(("proj", (wq_s, b2q, q_sb, 3)))
        work += [("av", jt) for jt in range(24, 32)]
        work += [("proj", (wq_s, b2q, q_sb, nh)) for nh in range(4, NIG)]
        wi = 0
        for jp in range(NJT // 2):
            issue_spair(1, jp)
            quota = (len(work) * (jp + 1)) // (NJT // 2)
            while wi < quota:
                kind, arg = work[wi]
                wi += 1
                if kind == "proj":
                    proj_group(*arg)
                elif kind == "tr":
                    dma_tr(arg)
                else:
                    issue_av(0, arg)
        issue_tree(1)

        pro.close()

        # epilogue-only pools (opened after x / the prologue PSUM are freed)
        otp = ctx.enter_context(tc.tile_pool(name="otp", bufs=2))
        yfp = ctx.enter_context(tc.tile_pool(name="yfp", bufs=2))
        ppf = ctx.enter_context(tc.tile_pool(name="ppf", bufs=2, space="PSUM"))

        issue_ot_op(0, otp, yfp)

        # ---- windows 2..7: scores(g) + AV(g-1) + output projection(g-1).
        # Group 7's AV additionally trails its own exps inside window 7. ----
        for g in range(2, NIG):
            pt_tiles[g] = ptp.tile([P, NJT, IG], bf16, name=f"pt{g}", tag="pt")
            po_tiles[g - 1] = ppo.tile([P, IG], f32, name=f"po{g-1}", tag="po")
            self_trail = g == NIG - 1
            if self_trail:
                po_tiles[g] = ppo.tile([P, IG], f32, name=f"po{g}", tag="po")
            for jp in range(NJT // 2):
                issue_spair(g, jp)
                issue_av(g - 1, 2 * jp)
                issue_av(g - 1, 2 * jp + 1)
                if self_trail and jp >= 2:
                    issue_av(g, 2 * (jp - 2))
                    issue_av(g, 2 * (jp - 2) + 1)
            if g == NIG - 1:
                # first half of the last group's den tree level 1 (tiles 0..15
                # are ready mid-window, so this costs nothing at the end)
                scr7 = treep.tile([P, 24, IG], bf16, name="scr7", tag="scr")
                vh = pt_tiles[g][:, 0:16, :].rearrange("p (a b) i -> p a b i", b=2)
                nc.vector.tensor_add(scr7[:, 0:8, :], vh[:, :, 0, :], vh[:, :, 1, :])
            issue_ot_op(g - 1, otp, yfp)
            if g < NIG - 1:
                issue_tree(g)

        # tail: last 4 AV tiles of group 7, the rest of its den tree level 1
        # (kept off the yf-copy path), its output projection, then the
        # remaining tree levels
        g = NIG - 1
        for jt in range(NJT - 4, NJT):
            issue_av(g, jt)
        vh = pt_tiles[g][:, 16:32, :].rearrange("p (a b) i -> p a b i", b=2)
        nc.vector.tensor_add(scr7[:, 8:16, :], vh[:, :, 0, :], vh[:, :, 1, :])
        issue_ot_op(g, otp, yfp)
        issue_tree_levels(g, scr7)

    nc.compile()
    return nc


def get_nc():
    if "nc" not in _NC_CACHE:
        _NC_CACHE["nc"] = _build_nc()
    return _NC_CACHE["nc"]


def make_in_maps(inputs):
    x = np.ascontiguousarray(np.asarray(inputs["x"], dtype=np.float32))
    wq = np.asarray(inputs["wq"], np.float32)
    wk = np.asarray(inputs["wk"], np.float32)
    wv = np.asarray(inputs["wv"], np.float32)
    bq = np.asarray(inputs["bq"], np.float32)
    bk = np.asarray(inputs["bk"], np.float32)
    bv = np.asarray(inputs["bv"], np.float32)
    wo = np.asarray(inputs["wo"], np.float32)
    gn_scale = np.asarray(inputs["gn_scale"], np.float32)
    gn_bias = np.asarray(inputs["gn_bias"], np.float32)

    in_maps = []
    for cid in range(8):
        b, h = divmod(cid, HEADS)
        sl = slice(h * HC, (h + 1) * HC)
        in_maps.append(
            {
                "xb": np.ascontiguousarray(x[b].reshape(C, N).astype(ml_dtypes.bfloat16)),
                "wqt": np.ascontiguousarray(wq[sl, :].T),
                "wkt": np.ascontiguousarray(wk[sl, :].T),
                "wvt": np.ascontiguousarray(wv[sl, :].T),
                "wot": np.ascontiguousarray(wo[:, sl].T),
                "bqh": np.ascontiguousarray(bq[sl].reshape(HC, 1)),
                "bkh": np.ascontiguousarray(bk[sl].reshape(HC, 1)),
                "bvh": np.ascontiguousarray(bv[sl].reshape(HC, 1)),
                "gns": np.ascontiguousarray(gn_scale.reshape(1, C)),
                "gnb": np.ascontiguousarray(gn_bias.reshape(1, C)),
            }
        )
    return in_maps


def assemble_output(inputs, results):
    x = np.asarray(inputs["x"], np.float32)
    bo = np.asarray(inputs["bo"], np.float32)
    y = x.reshape(2, C, N).astype(np.float32).copy()
    y += bo.reshape(1, C, 1)
    for cid in range(8):
        b = cid // HEADS
        r = results[cid]
        den = np.asarray(r["dden"], np.float32).sum(axis=0)  # [N]
        y[b] += np.asarray(r["yp"], np.float32) * (1.0 / den)[None, :]
    return y.reshape(2, C, 64, 64)


def run(inputs, trace=False):
    from concourse.bass_utils import run_bass_kernel_spmd

    nc = get_nc()
    in_maps = make_in_maps(inputs)
    res = run_bass_kernel_spmd(nc, in_maps, list(range(8)), trace=trace)
    return assemble_output(inputs, res.results), res


def kernel(**inputs):
    y, _ = run(inputs, trace=False)
    return y
